# revision 35
# baseline (speedup 1.0000x reference)
"""Trainium2 Bass kernel for the DocRED-style segment_reduce model.

Sharding: 8 cores, data-parallel: core c -> (doc = c//2, pair-half = c%2).
Each core independently computes logits for its 256 pairs. No collectives.
All segment reductions / gathers are lowered to one-hot matmuls whose
one-hot matrices are built on the host from the integer inputs and passed
as per-core input tensors (the SPMD program itself is index-agnostic).

P6 (block-bilinear classifier) uses a factored (i,j) chunk packing:
contraction chunk q=(c,m) has partition p <-> (i = 8c + p//16, j = 16m +
p%16), so each partition only needs 8 hs rows and 4 ts rows per block --
replicated by SBUF->SBUF broadcast DMA (no PE one-hot replication, no
PSUM->SBUF ACT copies).
"""

import os

import numpy as np

import concourse.bacc as bacc
import concourse.bass as bass
import concourse.mybir as mybir
import concourse.tile as tile
from concourse.bass_utils import run_bass_kernel_spmd

B, M, H = 4, 128, 1024
NH, L = 16, 1024
E, R = 64, 512
EMB, BS, NCL = 768, 64, 97
K12 = EMB // BS  # 12 blocks
NCORES = 8
RPC = R // 2  # pairs per core

# P6 packing: i = IG*c + p//16, j = JG_SZ*m + p%16 ; q = c*NJ + m
NI = 8   # number of i-groups (c)
NJ = 4   # number of j-groups (m)

F32 = mybir.dt.float32
BF16 = mybir.dt.bfloat16

MM_MODE = os.environ.get("DOCRED_MM_MODE", "bf16")
# 0..4: how many of the 4 head-groups per lc use the ACT-copy product path
P3_PATHA = int(os.environ.get("DOCRED_P3_PATHA", "2"))
# which ks of P6 run their bl products on gpsimd instead of DVE
P6_POOL_KS = set(
    int(x) for x in os.environ.get("DOCRED_P6_POOL_KS", "").split(",") if x
)


class _Builder:
    def __init__(self, mm_mode: str):
        self.mm_mode = mm_mode
        nc = bacc.Bacc("TRN2", target_bir_lowering=False, debug=False)
        self.nc = nc
        fdt = BF16
        d = {}
        d["ent"] = nc.dram_tensor("ent", [M, H], F32, kind="ExternalInput")
        d["attn"] = nc.dram_tensor("attn", [M, NH * L], fdt, kind="ExternalInput")
        d["seq"] = nc.dram_tensor("seq", [128, 8 * (L + 1)], fdt, kind="ExternalInput")
        d["ssum"] = nc.dram_tensor("ssum", [M, E], fdt, kind="ExternalInput")
        d["ohxy2"] = nc.dram_tensor("ohxy2", [M, 2 * RPC], fdt, kind="ExternalInput")
        d["eadd"] = nc.dram_tensor("eadd", [E, 1], F32, kind="ExternalInput")
        d["ohx"] = nc.dram_tensor("ohx", [E, RPC], fdt, kind="ExternalInput")
        d["ohy"] = nc.dram_tensor("ohy", [E, RPC], fdt, kind="ExternalInput")
        d["wh"] = nc.dram_tensor("wh", [128, 16 * EMB], fdt, kind="ExternalInput")
        d["wt"] = nc.dram_tensor("wt", [128, 16 * EMB], fdt, kind="ExternalInput")
        d["bh"] = nc.dram_tensor("bh", [128, EMB // 128], F32, kind="ExternalInput")
        d["bt"] = nc.dram_tensor("bt", [128, EMB // 128], F32, kind="ExternalInput")
        d["wb"] = nc.dram_tensor("wb", [128, 384 * NCL], fdt, kind="ExternalInput")
        d["bbc"] = nc.dram_tensor("bbc", [NCL, 1], F32, kind="ExternalInput")
        d["ident"] = nc.dram_tensor("ident", [128, 128], fdt, kind="ExternalInput")
        d["tsel"] = nc.dram_tensor("tsel", [128, 8 * 128], fdt,
                                   kind="ExternalInput")
        d["lt"] = nc.dram_tensor("lt", [NCL, RPC], F32, kind="ExternalOutput")
        self.debug = os.environ.get("DOCRED_DEBUG", "0") == "1"
        if self.debug:
            for nm, sh in (("dbg_ct", [128, 8 * RPC]),
                           ("dbg_hs", [128, 6 * RPC]),
                           ("dbg_ts", [128, 6 * RPC]),
                           ("dbg_hss", [128, NI * RPC]),
                           ("dbg_tss", [128, NJ * RPC]),
                           ("dbg_bl", [128, NI * NJ * RPC]),
                           ("dbg_rel", [128, 8 * RPC])):
                d[nm] = nc.dram_tensor(nm, sh, BF16, kind="ExternalOutput")
        self.d = d
        with tile.TileContext(nc) as tc:
            self.build(tc)
        nc.compile()

    def mm(self, out, lhsT, rhs, **kw):
        return self.nc.tensor.matmul(out, lhsT, rhs, **kw)

    def tp(self, out, in_, ident, **kw):
        return self.nc.tensor.matmul(out, in_, ident, is_transpose=True, **kw)

    def build(self, tc):
        nc = self.nc
        d = self.d
        fdt = BF16
        AF = mybir.ActivationFunctionType

        with (
            tc.tile_pool(name="pin", bufs=1) as pin,
            tc.tile_pool(name="mid", bufs=1) as mid,
            tc.tile_pool(name="gx", bufs=2) as gxpool,
        ):
            wbp_cm = tc.tile_pool(name="wbp", bufs=1)
            wbp = wbp_cm.__enter__()
            NWB = 8  # chunks preloaded early; the rest stream at P6 start
            wb_sb = wbp.tile([128, NWB, 32 * NCL], fdt, name="wb_sb")
            wpool_cm = tc.tile_pool(name="wpool", bufs=1)
            wpool = wpool_cm.__enter__()
            # ---------- persistent tiles ----------
            ident = pin.tile([128, 128], fdt)
            ohxy2 = pin.tile([M, 2 * RPC], fdt)
            ssum = pin.tile([M, E], fdt)
            eadd = pin.tile([E, 1], F32)
            ohx = pin.tile([E, RPC], fdt)
            ohy = pin.tile([E, RPC], fdt)
            bh = pin.tile([128, EMB // 128], F32)
            bt = pin.tile([128, EMB // 128], F32)
            bbc = pin.tile([NCL, 1], F32)
            tsel = pin.tile([128, 8, 128], fdt)
            seq_sb = pin.tile([128, 8, L + 1], fdt)
            wh_sb = wpool.tile([128, 16, EMB], fdt, name="wh_sb")
            wt_sb = wpool.tile([128, 16, EMB], fdt, name="wt_sb")
            # wb chunks streamed (4-deep) during P6
            hsEt = mid.tile([128, 6, RPC], fdt, name="hsEt")
            tsEt = mid.tile([128, 6, RPC], fdt, name="tsEt")
            CTmm = mid.tile([128, 8, RPC], fdt, name="CTmm")
            relT = mid.tile([128, 8, RPC], fdt, name="relT")

            # ---------- DMA prefetch (sync queue, in priority order) -------
            for t, key in [
                (ident, "ident"), (ohxy2, "ohxy2"), (ssum, "ssum"),
                (eadd, "eadd"), (ohx, "ohx"), (ohy, "ohy"),
                (bh, "bh"), (bt, "bt"), (bbc, "bbc"),
            ]:
                nc.sync.dma_start(t[:], d[key].ap())
            nc.sync.dma_start(
                tsel[:], d["tsel"].ap().rearrange("p (a b) -> p a b", a=8))

            with (
                tc.tile_pool(name="ahpool", bufs=1) as ahpool,
                tc.tile_pool(name="entp", bufs=1) as entp,
            ):
                attn = ahpool.tile([M, NH, L], fdt)
                av = d["attn"].ap().rearrange("p (h l) -> p h l", h=NH)
                for q in range(4):
                    nc.sync.dma_start(attn[:, 4 * q:4 * (q + 1), :],
                                      av[:, 4 * q:4 * (q + 1), :])
                ent = entp.tile([M, H], F32)
                nc.sync.dma_start(ent[:], d["ent"].ap())
                nc.sync.dma_start(
                    wh_sb[:], d["wh"].ap().rearrange("p (a b) -> p a b", a=16))
                nc.sync.dma_start(
                    wt_sb[:], d["wt"].ap().rearrange("p (a b) -> p a b", a=16))
                nc.sync.dma_start(
                    seq_sb[:], d["seq"].ap().rearrange("p (a b) -> p a b", a=8))

                # ---------- P1: exp + segment-sum + log ----------
                psA_cm = tc.tile_pool(name="psA", bufs=1, space="PSUM")
                psA = psA_cm.__enter__()
                pexp = entp.tile([M, H], fdt, name="pexp")
                nc.scalar.activation(pexp[:], ent[:], AF.Exp)
                ps_ent = psA.tile([E, H], F32)
                for nh in range(2):
                    self.mm(ps_ent[:, nh * 512:(nh + 1) * 512], ssum[:],
                            pexp[:, nh * 512:(nh + 1) * 512])
                ent_sb = mid.tile([E, H], fdt)
                nc.scalar.activation(ent_sb[:], ps_ent[:], AF.Ln, bias=eadd[:])
                psA_cm.__exit__(None, None, None)

                # ---------- P3: C = sum_h gather_x(attn_h)*gather_y(attn_h)
                psC_cm = tc.tile_pool(name="psC", bufs=2, space="PSUM")
                psC = psC_cm.__enter__()
                for lc in range(8):
                    tgs = []
                    for hg in range(4):
                        ps_g = psC.tile([128, 4, 2, RPC], F32, tag="g",
                                        name="ps_g")
                        for h in range(4):
                            a_sl = attn[:, 4 * hg + h, lc * 128:(lc + 1) * 128]
                            self.mm(ps_g[:, h], a_sl, ohxy2[:])
                        tg = gxpool.tile([128, 4, RPC], fdt, tag=f"tg{hg}",
                                         bufs=2, name=f"tg{hg}")
                        if hg < P3_PATHA:
                            # path A: ACT copies both X and Y -> 2x DVE mult
                            xy = gxpool.tile([128, 4, 2, RPC], fdt, tag="xy",
                                             bufs=1, name="xy")
                            nc.scalar.copy(xy[:], ps_g[:])
                            nc.vector.tensor_mul(tg[:], xy[:, :, 0, :],
                                                 xy[:, :, 1, :])
                        else:
                            # path B: ACT copies X only -> 1x DVE mult vs PSUM
                            xh = gxpool.tile([128, 4, RPC], fdt, tag="xh",
                                             bufs=2, name="xh")
                            nc.scalar.copy(xh[:], ps_g[:, :, 0, :])
                            nc.vector.tensor_mul(tg[:], xh[:],
                                                 ps_g[:, :, 1, :])
                        tgs.append(tg)
                    # tree reduce 4x[128,4,256] -> CTmm[:, lc, :]
                    nc.vector.tensor_add(tgs[0][:], tgs[0][:], tgs[1][:])
                    nc.vector.tensor_add(tgs[2][:], tgs[2][:], tgs[3][:])
                    nc.vector.tensor_add(tgs[0][:], tgs[0][:], tgs[2][:])
                    nc.vector.tensor_add(tgs[0][:, 0:2, :], tgs[0][:, 0:2, :],
                                         tgs[0][:, 2:4, :])
                    nc.vector.tensor_add(CTmm[:, lc, :], tgs[0][:, 0, :],
                                         tgs[0][:, 1, :])
                psC_cm.__exit__(None, None, None)

            # attn pool freed here. Prefetch ALL of wb now on the idle sync
            # queue (no waits -> transfers run during EW/P4/P5 and finish
            # before the classifier needs them).
            wbv = d["wb"].ap().rearrange("p (k x) -> p k x", k=K12)
            for k in range(NWB):
                nc.sync.dma_start(wb_sb[:, k, :], wbv[:, k, :])

            # ---------- EW = ent @ W[0:1024] + entT transposes ----------
            psEW_cm = tc.tile_pool(name="psEW", bufs=1, space="PSUM")
            psEW = psEW_cm.__enter__()
            entT = mid.tile([128, 8, E], fdt, name="entT")
            for hc in range(8):
                ps_t2 = psEW.tile([128, E], fdt, tag="sm", name="ps_t2",
                                  bufs=2)
                self.tp(ps_t2[:], ent_sb[:, hc * 128:(hc + 1) * 128],
                        ident[0:E, 0:E])
                nc.scalar.copy(entT[:, hc, :], ps_t2[:])
            EWh = mid.tile([E, EMB], fdt, name="EWh")
            EWt = mid.tile([E, EMB], fdt, name="EWt")
            for w, ew in ((wh_sb, EWh), (wt_sb, EWt)):
                ps_ew = psEW.tile([E, EMB], F32, tag="ew", name="ps_ew")
                for hc in range(8):
                    for lo, hi in ((0, 512), (512, 768)):
                        self.mm(ps_ew[:, lo:hi], entT[:, hc, :],
                                w[:, hc, lo:hi],
                                start=(hc == 0), stop=(hc == 7))
                nc.scalar.copy(ew[:], ps_ew[:])
            psEW_cm.__exit__(None, None, None)

            # ---------- P4: rel = normalize(C) @ seq, transpose ----------
            psR_cm = tc.tile_pool(name="psR", bufs=2, space="PSUM")
            psR = psR_cm.__enter__()
            ps_rel = [psR.tile([128, L], F32, tag="big", name=f"ps_rel{i}")
                      for i in range(2)]
            ps_s8 = psR.tile([128, 2, 8], F32, tag="ss", name="ps_s8", bufs=1)
            for lc in range(8):
                st, sp = lc == 0, lc == 7
                for rc in range(2):
                    lhsT = CTmm[:, lc, rc * 128:(rc + 1) * 128]
                    self.mm(ps_rel[rc][:, 0:512], lhsT, seq_sb[:, lc, 0:512],
                            start=st, stop=sp)
                    self.mm(ps_rel[rc][:, 512:1024], lhsT,
                            seq_sb[:, lc, 512:1024], start=st, stop=sp)
                    self.mm(ps_s8[:, rc, lc:lc + 1], lhsT,
                            seq_sb[:, lc, 1024:1025], start=True, stop=True)
            for rc in range(2):
                tdenom = gxpool.tile([128, 1], F32, tag="tden")
                nc.vector.tensor_reduce(tdenom[:], ps_s8[:, rc, :],
                                        axis=mybir.AxisListType.X,
                                        op=mybir.AluOpType.add)
                nc.scalar.activation(tdenom[:], tdenom[:], AF.Copy,
                                     bias=16e-5, scale=1.0)
                frec = gxpool.tile([128, 1], F32, tag="frec")
                nc.vector.reciprocal(frec[:], tdenom[:])
                rel_sc = mid.tile([128, L], fdt, tag="rel_sc", name="rel_sc")
                nc.vector.tensor_scalar_mul(rel_sc[:], ps_rel[rc][:], frec[:])
                for dc in range(8):
                    ps_t = psR.tile([128, 128], fdt, tag="sm", name="ps_t")
                    self.tp(ps_t[:], rel_sc[:, dc * 128:(dc + 1) * 128],
                            ident[:])
                    nc.scalar.copy(relT[:, dc, rc * 128:(rc + 1) * 128],
                                   ps_t[:])
            psR_cm.__exit__(None, None, None)

            # ---------- P5: extractors -> hsEt/tsEt [emb, n] ----------
            psE_cm = tc.tile_pool(name="psE", bufs=3, space="PSUM")
            psE = psE_cm.__enter__()
            drh_cm = tc.tile_pool(name="drh", bufs=6, space="DRAM")
            drh = drh_cm.__enter__()
            hsDs = []
            for ec in range(6):
                for (w, bvec, ew, oh, dst) in (
                    (wh_sb, bh, EWh, ohx, hsEt),
                    (wt_sb, bt, EWt, ohy, tsEt),
                ):
                    ps_e = psE.tile([128, RPC], F32, tag="sm", name="ps_e")
                    self.mm(ps_e[:], ew[:, ec * 128:(ec + 1) * 128], oh[:],
                            start=True, stop=False)
                    for kc in range(8, 16):
                        self.mm(ps_e[:], w[:, kc, ec * 128:(ec + 1) * 128],
                                relT[:, kc % 8, :],
                                start=False, stop=(kc == 15))
                    nc.scalar.activation(dst[:, ec, :], ps_e[:], AF.Tanh,
                                         bias=bvec[:, ec:ec + 1])
                # stage this hs chunk to DRAM for the P6 broadcast chain
                hsD = drh.tile([128, RPC], fdt, tag="hsD", name=f"hsD{ec}")
                nc.sync.dma_start(hsD[:], hsEt[:, ec, :])
                hsDs.append(hsD)
            psE_cm.__exit__(None, None, None)
            wpool_cm.__exit__(None, None, None)

            # ---------- P6: factored bl products + classifier GEMM -------
            with (
                tc.tile_pool(name="stp", bufs=2) as stp,
                tc.tile_pool(name="blp", bufs=2) as blp,
                tc.tile_pool(name="dramp", bufs=2, space="DRAM") as dramp,
                tc.tile_pool(name="psTS", bufs=2, space="PSUM") as psTS,
                tc.tile_pool(name="ps_lt", bufs=1, space="PSUM") as ps_lt,
            ):
                pslt = ps_lt.tile([NCL, RPC], F32)
                wb_late = []
                for k in range(NWB, K12):
                    wbl = blp.tile([128, 32 * NCL], fdt, tag=f"wbl{k}",
                                   bufs=1, name=f"wbl{k}")
                    nc.sync.dma_start(wbl[:], wbv[:, k, :])
                    wb_late.append(wbl)
                for k in range(K12):
                    ec, kk = k // 2, 64 * (k % 2)
                    hsS = stp.tile([128, NI, RPC], fdt, tag="hs", name="hsS")
                    tsS = stp.tile([128, NJ, RPC], fdt, tag="ts", name="tsS")
                    # hs: D2D broadcast replicate (gpsimd) + plain read
                    # (scalar).  hsS[p, c, :] = hs[kk + 8*(p%8) + c]
                    hsR = dramp.tile([128, NI * RPC], fdt, tag="hsR",
                                     name="hsR")
                    nc.gpsimd.dma_start(
                        hsR[:].rearrange("(j a) x -> j a x", j=16),
                        hsDs[ec][kk:kk + 64, :]
                        .rearrange("(a c) n -> a (c n)", a=8)
                        .unsqueeze(0).broadcast_to([16, 8, NI * RPC]))
                    nc.scalar.dma_start(
                        hsS[:], hsR[:].rearrange("p (c n) -> p c n", c=NI))
                    # ts: PE one-hot replication + ACT copy.
                    # tsS[p, m, :] = ts[kk + 4*(p//8) + m]
                    tsPS = psTS.tile([128, NJ, RPC], F32, tag="tp",
                                     name="tsPS")
                    for m in range(NJ):
                        self.mm(tsPS[:, m, :],
                                tsel[:, 4 * (k % 2) + m, :],
                                tsEt[:, ec, :])
                    nc.scalar.copy(tsS[:], tsPS[:])
                    if self.debug and k == 0:
                        nc.scalar.dma_start(d["dbg_hss"].ap(), hsS[:])
                        nc.scalar.dma_start(d["dbg_tss"].ap(), tsS[:])
                    blT = blp.tile([128, NI, NJ, RPC], fdt, tag="blT",
                                   name="blT")
                    in0 = hsS[:].unsqueeze(2).broadcast_to([128, NI, NJ, RPC])
                    in1 = tsS[:].unsqueeze(1).broadcast_to([128, NI, NJ, RPC])
                    if k in P6_POOL_KS:
                        nc.gpsimd.scalar_tensor_tensor(
                            blT[:], in0, 1.0, in1,
                            op0=mybir.AluOpType.mult, op1=mybir.AluOpType.mult)
                    else:
                        for ch in range(2):
                            nc.vector.tensor_mul(
                                blT[:, 4 * ch:4 * (ch + 1), :, :],
                                in0[:, 4 * ch:4 * (ch + 1), :, :],
                                in1[:, 4 * ch:4 * (ch + 1), :, :])
                    if self.debug and k == 0:
                        nc.scalar.dma_start(d["dbg_bl"].ap(), blT[:])
                    for c in range(NI):
                        for m in range(NJ):
                            q = c * NJ + m
                            cg = k * 32 + q
                            wbk = (wb_sb[:, k, q * NCL:(q + 1) * NCL]
                                   if k < NWB else
                                   wb_late[k - NWB][:, q * NCL:(q + 1) * NCL])
                            self.mm(pslt[:], wbk, blT[:, c, m, :],
                                    start=(cg == 0), stop=(cg == 383))

                out_sb = mid.tile([NCL, RPC], F32, name="out_sb")
                nc.scalar.activation(out_sb[:], pslt[:], AF.Identity,
                                     bias=bbc[:])
                nc.sync.dma_start(d["lt"].ap(), out_sb[:])
                if self.debug:
                    nc.sync.dma_start(d["dbg_ct"].ap(), CTmm[:])
                    nc.sync.dma_start(d["dbg_hs"].ap(), hsEt[:])
                    nc.sync.dma_start(d["dbg_ts"].ap(), tsEt[:])
                    nc.sync.dma_start(d["dbg_rel"].ap(), relT[:])
            drh_cm.__exit__(None, None, None)
            wbp_cm.__exit__(None, None, None)


_PROGRAM_CACHE = {}


def _get_program(mm_mode: str):
    if mm_mode not in _PROGRAM_CACHE:
        _PROGRAM_CACHE[mm_mode] = _Builder(mm_mode)
    return _PROGRAM_CACHE[mm_mode]


def _host_inputs(seq_lhs, ent_lhs, ent_to_seq_attn, entity_id_labels, hts,
                 Wh, bh, Wt, bt, Wb, bb):
    """Build the 8 per-core input maps (all host-side numpy)."""
    import ml_dtypes

    fdt = np.dtype(ml_dtypes.bfloat16)
    seq_lhs = np.asarray(seq_lhs, np.float32)
    ent_lhs = np.asarray(ent_lhs, np.float32)
    ent_to_seq_attn = np.asarray(ent_to_seq_attn, np.float32)
    entity_id_labels = np.asarray(entity_id_labels)
    hts = np.asarray(hts)
    Wh = np.asarray(Wh, np.float32)
    Wt = np.asarray(Wt, np.float32)
    Wb = np.asarray(Wb, np.float32)
    bh = np.asarray(bh, np.float32)
    bt = np.asarray(bt, np.float32)
    bb = np.asarray(bb, np.float32)

    # device chunk (k, q=(c,m)) row p maps to Wb row k*4096 + 64*i + j with
    # i = 8c + p//16, j = 16m + p%16
    p_ = np.arange(128)
    c_ = np.arange(NI)
    m_ = np.arange(NJ)
    k_ = np.arange(K12)
    i_ = (8 * (p_[None, None, None, :] % 8)
          + c_[None, :, None, None])                  # [1, c, 1, p]
    j_ = (4 * (p_[None, None, None, :] // 8)
          + m_[None, None, :, None])                  # [1, 1, m, p]
    rows = (k_[:, None, None, None] * 4096 + 64 * i_ + j_)  # [k, c, m, p]
    wb_r = np.ascontiguousarray(
        Wb[rows.reshape(-1), :].reshape(K12 * 32, 128, NCL)
        .transpose(1, 0, 2).reshape(128, 384 * NCL)
    ).astype(fdt)
    wh_c = np.ascontiguousarray(
        Wh.reshape(16, 128, EMB).transpose(1, 0, 2).reshape(128, 16 * EMB)
    ).astype(fdt)
    wt_c = np.ascontiguousarray(
        Wt.reshape(16, 128, EMB).transpose(1, 0, 2).reshape(128, 16 * EMB)
    ).astype(fdt)
    bh_c = np.ascontiguousarray(bh.reshape(EMB // 128, 128).T)
    bt_c = np.ascontiguousarray(bt.reshape(EMB // 128, 128).T)
    bb_c = np.ascontiguousarray(bb.reshape(NCL, 1))
    ident = np.eye(128, dtype=np.float32).astype(fdt)
    # tsel[r, 4*par + m, p] = 1 iff r == 64*par + 4*(p//8) + m
    tsel_h = np.zeros((128, 8, 128), np.float32)
    par_ = np.arange(2)
    r_ = (64 * par_[:, None, None] + 4 * (p_[None, None, :] // 8)
          + m_[None, :, None])                        # [par, m, p]
    s_ = 4 * par_[:, None, None] + m_[None, :, None]  # [par, m, 1]
    tsel_h[r_.reshape(-1), np.broadcast_to(s_, r_.shape).reshape(-1),
           np.broadcast_to(p_[None, None, :], r_.shape).reshape(-1)] = 1.0
    tsel_h = tsel_h.reshape(128, 8 * 128).astype(fdt)

    in_maps = []
    for c in range(NCORES):
        doc, half = divmod(c, 2)
        sl = slice(half * RPC, (half + 1) * RPC)
        labels = entity_id_labels[doc].astype(np.int64)
        cnt = np.bincount(labels, minlength=E).astype(np.float32)
        S = np.zeros((M, E), np.float32)
        S[np.arange(M), labels] = 1.0
        smean = S / np.maximum(cnt, 1.0)[None, :]  # [M, E]
        eadd = (cnt == 0).astype(np.float32).reshape(E, 1)
        hi = hts[doc, sl, 0].astype(np.int64)
        ti = hts[doc, sl, 1].astype(np.int64)
        ohx = np.zeros((E, RPC), np.float32)
        ohx[hi, np.arange(RPC)] = 1.0
        ohy = np.zeros((E, RPC), np.float32)
        ohy[ti, np.arange(RPC)] = 1.0
        ohxy2 = np.concatenate([smean @ ohx, smean @ ohy], axis=1)  # [M, 512]
        attn = np.ascontiguousarray(
            ent_to_seq_attn[doc].transpose(1, 0, 2).reshape(M, NH * L)
        ).astype(fdt)
        seq_r = seq_lhs[doc].reshape(8, 128, L).transpose(1, 0, 2)
        seq_aug = np.concatenate(
            [seq_r, np.ones((128, 8, 1), np.float32)], axis=2
        )
        in_maps.append({
            "ent": np.ascontiguousarray(ent_lhs[doc]),
            "attn": attn,
            "seq": np.ascontiguousarray(
                seq_aug.reshape(128, 8 * (L + 1))).astype(fdt),
            "ssum": S.astype(fdt),
            "ohxy2": ohxy2.astype(fdt),
            "eadd": eadd,
            "ohx": ohx.astype(fdt),
            "ohy": ohy.astype(fdt),
            "wh": wh_c, "wt": wt_c, "bh": bh_c, "bt": bt_c,
            "wb": wb_r, "bbc": bb_c, "ident": ident, "tsel": tsel_h,
        })
    return in_maps


_LAST_RESULTS = {}


def kernel(**inputs) -> np.ndarray:
    prog = _get_program(MM_MODE)
    in_maps = _host_inputs(**inputs)
    trace = os.environ.get("DOCRED_TRACE", "0") == "1"
    res = run_bass_kernel_spmd(
        prog.nc, in_maps, core_ids=list(range(NCORES)), trace=trace,
    )
    _LAST_RESULTS["res"] = res
    out = np.empty((B * R, NCL), np.float32)
    for c in range(NCORES):
        doc, half = divmod(c, 2)
        lt = res.results[c]["lt"]  # [NCL, RPC]
        out[doc * R + half * RPC: doc * R + (half + 1) * RPC, :] = lt.T
    return out


# revision 36
# speedup vs baseline: 1.1718x; 1.1718x over previous
"""Trainium2 Bass kernel for the DocRED-style segment_reduce model.

Sharding: 8 cores, data-parallel: core c -> (doc = c//2, pair-half = c%2).
Each core independently computes logits for its 256 pairs. No collectives.
All segment reductions / gathers are lowered to one-hot matmuls whose
one-hot matrices are built on the host from the integer inputs and passed
as per-core input tensors (the SPMD program itself is index-agnostic).

P6 (block-bilinear classifier) uses a factored (i,j) chunk packing:
contraction chunk q=(c,m) has partition p <-> (i = 8c + p//16, j = 16m +
p%16), so each partition only needs 8 hs rows and 4 ts rows per block --
replicated by SBUF->SBUF broadcast DMA (no PE one-hot replication, no
PSUM->SBUF ACT copies).
"""

import os

import numpy as np

import concourse.bacc as bacc
import concourse.bass as bass
import concourse.mybir as mybir
import concourse.tile as tile
from concourse.bass_utils import run_bass_kernel_spmd

B, M, H = 4, 128, 1024
NH, L = 16, 1024
E, R = 64, 512
EMB, BS, NCL = 768, 64, 97
K12 = EMB // BS  # 12 blocks
NCORES = 8
RPC = R // 2  # pairs per core

# P6 packing: i = IG*c + p//16, j = JG_SZ*m + p%16 ; q = c*NJ + m
NI = 8   # number of i-groups (c)
NJ = 4   # number of j-groups (m)

F32 = mybir.dt.float32
BF16 = mybir.dt.bfloat16

MM_MODE = os.environ.get("DOCRED_MM_MODE", "bf16")
# 0..4: how many of the 4 head-groups per lc use the ACT-copy product path
P3_PATHA = int(os.environ.get("DOCRED_P3_PATHA", "2"))
# which ks of P6 run their bl products on gpsimd instead of DVE
P6_POOL_KS = set(
    int(x) for x in os.environ.get("DOCRED_P6_POOL_KS", "").split(",") if x
)


class _Builder:
    def __init__(self, mm_mode: str):
        self.mm_mode = mm_mode
        nc = bacc.Bacc("TRN2", target_bir_lowering=False, debug=False)
        self.nc = nc
        fdt = BF16
        d = {}
        d["ent"] = nc.dram_tensor("ent", [M, H], F32, kind="ExternalInput")
        d["attn"] = nc.dram_tensor("attn", [M, NH * L], fdt, kind="ExternalInput")
        d["seq"] = nc.dram_tensor("seq", [128, 8 * (L + 1)], fdt, kind="ExternalInput")
        d["ssum"] = nc.dram_tensor("ssum", [M, E], fdt, kind="ExternalInput")
        d["ohxy2"] = nc.dram_tensor("ohxy2", [M, 2 * RPC], fdt, kind="ExternalInput")
        d["eadd"] = nc.dram_tensor("eadd", [E, 1], F32, kind="ExternalInput")
        d["ohx"] = nc.dram_tensor("ohx", [E, RPC], fdt, kind="ExternalInput")
        d["ohy"] = nc.dram_tensor("ohy", [E, RPC], fdt, kind="ExternalInput")
        d["wh"] = nc.dram_tensor("wh", [128, 16 * EMB], fdt, kind="ExternalInput")
        d["wt"] = nc.dram_tensor("wt", [128, 16 * EMB], fdt, kind="ExternalInput")
        d["bh"] = nc.dram_tensor("bh", [128, EMB // 128], F32, kind="ExternalInput")
        d["bt"] = nc.dram_tensor("bt", [128, EMB // 128], F32, kind="ExternalInput")
        d["wb"] = nc.dram_tensor("wb", [128, 384 * NCL], fdt, kind="ExternalInput")
        d["bbc"] = nc.dram_tensor("bbc", [NCL, 1], F32, kind="ExternalInput")
        d["ident"] = nc.dram_tensor("ident", [128, 128], fdt, kind="ExternalInput")
        d["lt"] = nc.dram_tensor("lt", [NCL, RPC], F32, kind="ExternalOutput")
        self.debug = os.environ.get("DOCRED_DEBUG", "0") == "1"
        if self.debug:
            for nm, sh in (("dbg_ct", [128, 8 * RPC]),
                           ("dbg_hs", [128, 6 * RPC]),
                           ("dbg_ts", [128, 6 * RPC]),
                           ("dbg_hss", [128, NI * RPC]),
                           ("dbg_tss", [128, NJ * RPC]),
                           ("dbg_bl", [128, NI * NJ * RPC]),
                           ("dbg_rel", [128, 8 * RPC])):
                d[nm] = nc.dram_tensor(nm, sh, BF16, kind="ExternalOutput")
        self.d = d
        with tile.TileContext(nc) as tc:
            self.build(tc)
        nc.compile()

    def mm(self, out, lhsT, rhs, **kw):
        return self.nc.tensor.matmul(out, lhsT, rhs, **kw)

    def tp(self, out, in_, ident, **kw):
        return self.nc.tensor.matmul(out, in_, ident, is_transpose=True, **kw)

    def build(self, tc):
        nc = self.nc
        d = self.d
        fdt = BF16
        AF = mybir.ActivationFunctionType

        with (
            tc.tile_pool(name="pin", bufs=1) as pin,
            tc.tile_pool(name="mid", bufs=1) as mid,
            tc.tile_pool(name="gx", bufs=2) as gxpool,
        ):
            wbp_cm = tc.tile_pool(name="wbp", bufs=1)
            wbp = wbp_cm.__enter__()
            NWB = 7  # chunks preloaded early; the rest stream at P6 start
            wb_sb = wbp.tile([128, NWB, 32 * NCL], fdt, name="wb_sb")
            wpool_cm = tc.tile_pool(name="wpool", bufs=1)
            wpool = wpool_cm.__enter__()
            # ---------- persistent tiles ----------
            ident = pin.tile([128, 128], fdt)
            ohxy2 = pin.tile([M, 2 * RPC], fdt)
            ssum = pin.tile([M, E], fdt)
            eadd = pin.tile([E, 1], F32)
            ohx = pin.tile([E, RPC], fdt)
            ohy = pin.tile([E, RPC], fdt)
            bh = pin.tile([128, EMB // 128], F32)
            bt = pin.tile([128, EMB // 128], F32)
            bbc = pin.tile([NCL, 1], F32)
            seq_sb = pin.tile([128, 8, L + 1], fdt)
            wh_sb = wpool.tile([128, 16, EMB], fdt, name="wh_sb")
            wt_sb = wpool.tile([128, 16, EMB], fdt, name="wt_sb")
            # wb chunks streamed (4-deep) during P6
            hsEt = mid.tile([128, 6, RPC], fdt, name="hsEt")
            tsEt = mid.tile([128, 6, RPC], fdt, name="tsEt")
            CTmm = mid.tile([128, 8, RPC], fdt, name="CTmm")
            relT = mid.tile([128, 8, RPC], fdt, name="relT")

            # ---------- DMA prefetch (sync queue, in priority order) -------
            for t, key in [
                (ident, "ident"), (ohxy2, "ohxy2"), (ssum, "ssum"),
                (eadd, "eadd"), (ohx, "ohx"), (ohy, "ohy"),
                (bh, "bh"), (bt, "bt"), (bbc, "bbc"),
            ]:
                nc.sync.dma_start(t[:], d[key].ap())

            with (
                tc.tile_pool(name="ahpool", bufs=1) as ahpool,
                tc.tile_pool(name="entp", bufs=1) as entp,
            ):
                ent = entp.tile([M, H], F32)
                nc.sync.dma_start(ent[:], d["ent"].ap())
                attn = ahpool.tile([M, NH, L], fdt)
                av = d["attn"].ap().rearrange("p (h l) -> p h l", h=NH)
                for q in range(4):
                    nc.sync.dma_start(attn[:, 4 * q:4 * (q + 1), :],
                                      av[:, 4 * q:4 * (q + 1), :])
                nc.sync.dma_start(
                    wh_sb[:], d["wh"].ap().rearrange("p (a b) -> p a b", a=16))
                nc.sync.dma_start(
                    wt_sb[:], d["wt"].ap().rearrange("p (a b) -> p a b", a=16))
                nc.sync.dma_start(
                    seq_sb[:], d["seq"].ap().rearrange("p (a b) -> p a b", a=8))

                # ---------- P1: exp + segment-sum + log ----------
                psA_cm = tc.tile_pool(name="psA", bufs=1, space="PSUM")
                psA = psA_cm.__enter__()
                pexp = entp.tile([M, H], fdt, name="pexp")
                nc.scalar.activation(pexp[:], ent[:], AF.Exp)
                ps_ent = psA.tile([E, H], F32)
                for nh in range(2):
                    self.mm(ps_ent[:, nh * 512:(nh + 1) * 512], ssum[:],
                            pexp[:, nh * 512:(nh + 1) * 512])
                ent_sb = mid.tile([E, H], fdt)
                nc.scalar.activation(ent_sb[:], ps_ent[:], AF.Ln, bias=eadd[:])
                psA_cm.__exit__(None, None, None)

                # ---------- P3: C = sum_h gather_x(attn_h)*gather_y(attn_h)
                psC_cm = tc.tile_pool(name="psC", bufs=2, space="PSUM")
                psC = psC_cm.__enter__()
                for lc in range(8):
                    tgs = []
                    for hg in range(4):
                        ps_g = psC.tile([128, 4, 2, RPC], F32, tag="g",
                                        name="ps_g")
                        for h in range(4):
                            a_sl = attn[:, 4 * hg + h, lc * 128:(lc + 1) * 128]
                            self.mm(ps_g[:, h], a_sl, ohxy2[:])
                        tg = gxpool.tile([128, 4, RPC], fdt, tag=f"tg{hg}",
                                         bufs=2, name=f"tg{hg}")
                        if hg < P3_PATHA:
                            # path A: ACT copies both X and Y -> 2x DVE mult
                            xy = gxpool.tile([128, 4, 2, RPC], fdt, tag="xy",
                                             bufs=2, name="xy")
                            nc.scalar.copy(xy[:], ps_g[:])
                            nc.vector.tensor_mul(tg[:], xy[:, :, 0, :],
                                                 xy[:, :, 1, :])
                        else:
                            # path B: ACT copies X only -> 1x DVE mult vs PSUM
                            xh = gxpool.tile([128, 4, RPC], fdt, tag="xh",
                                             bufs=2, name="xh")
                            nc.scalar.copy(xh[:], ps_g[:, :, 0, :])
                            nc.vector.tensor_mul(tg[:], xh[:],
                                                 ps_g[:, :, 1, :])
                        tgs.append(tg)
                    # tree reduce 4x[128,4,256] -> CTmm[:, lc, :]
                    nc.vector.tensor_add(tgs[0][:], tgs[0][:], tgs[1][:])
                    nc.vector.tensor_add(tgs[2][:], tgs[2][:], tgs[3][:])
                    nc.vector.tensor_add(tgs[0][:], tgs[0][:], tgs[2][:])
                    nc.vector.tensor_add(tgs[0][:, 0:2, :], tgs[0][:, 0:2, :],
                                         tgs[0][:, 2:4, :])
                    nc.vector.tensor_add(CTmm[:, lc, :], tgs[0][:, 0, :],
                                         tgs[0][:, 1, :])
                psC_cm.__exit__(None, None, None)

            # attn pool freed here. Prefetch ALL of wb now on the idle sync
            # queue (no waits -> transfers run during EW/P4/P5 and finish
            # before the classifier needs them).
            wbv = d["wb"].ap().rearrange("p (k x) -> p k x", k=K12)
            for k in range(NWB):
                nc.sync.dma_start(wb_sb[:, k, :], wbv[:, k, :])

            # ---------- EW = ent @ W[0:1024] + entT transposes ----------
            psEW_cm = tc.tile_pool(name="psEW", bufs=1, space="PSUM")
            psEW = psEW_cm.__enter__()
            entT = mid.tile([128, 8, E], fdt, name="entT")
            for hc in range(8):
                ps_t2 = psEW.tile([128, E], fdt, tag="sm", name="ps_t2",
                                  bufs=2)
                self.tp(ps_t2[:], ent_sb[:, hc * 128:(hc + 1) * 128],
                        ident[0:E, 0:E])
                nc.scalar.copy(entT[:, hc, :], ps_t2[:])
            EWh = mid.tile([E, EMB], fdt, name="EWh")
            EWt = mid.tile([E, EMB], fdt, name="EWt")
            for w, ew in ((wh_sb, EWh), (wt_sb, EWt)):
                ps_ew = psEW.tile([E, EMB], F32, tag="ew", name="ps_ew")
                for hc in range(8):
                    for lo, hi in ((0, 512), (512, 768)):
                        self.mm(ps_ew[:, lo:hi], entT[:, hc, :],
                                w[:, hc, lo:hi],
                                start=(hc == 0), stop=(hc == 7))
                nc.scalar.copy(ew[:], ps_ew[:])
            psEW_cm.__exit__(None, None, None)

            # ---------- P4: rel = normalize(C) @ seq, transpose ----------
            psR_cm = tc.tile_pool(name="psR", bufs=2, space="PSUM")
            psR = psR_cm.__enter__()
            ps_rel = [psR.tile([128, L], F32, tag="big", name=f"ps_rel{i}")
                      for i in range(2)]
            ps_s8 = psR.tile([128, 2, 8], F32, tag="ss", name="ps_s8", bufs=1)
            for lc in range(8):
                st, sp = lc == 0, lc == 7
                for rc in range(2):
                    lhsT = CTmm[:, lc, rc * 128:(rc + 1) * 128]
                    self.mm(ps_rel[rc][:, 0:512], lhsT, seq_sb[:, lc, 0:512],
                            start=st, stop=sp)
                    self.mm(ps_rel[rc][:, 512:1024], lhsT,
                            seq_sb[:, lc, 512:1024], start=st, stop=sp)
                    self.mm(ps_s8[:, rc, lc:lc + 1], lhsT,
                            seq_sb[:, lc, 1024:1025], start=True, stop=True)
            for rc in range(2):
                tdenom = gxpool.tile([128, 1], F32, tag="tden")
                nc.vector.tensor_reduce(tdenom[:], ps_s8[:, rc, :],
                                        axis=mybir.AxisListType.X,
                                        op=mybir.AluOpType.add)
                nc.scalar.activation(tdenom[:], tdenom[:], AF.Copy,
                                     bias=16e-5, scale=1.0)
                frec = gxpool.tile([128, 1], F32, tag="frec")
                nc.vector.reciprocal(frec[:], tdenom[:])
                rel_sc = mid.tile([128, L], fdt, tag="rel_sc", name="rel_sc")
                nc.vector.tensor_scalar_mul(rel_sc[:], ps_rel[rc][:], frec[:])
                for dc in range(8):
                    ps_t = psR.tile([128, 128], fdt, tag="sm", name="ps_t")
                    self.tp(ps_t[:], rel_sc[:, dc * 128:(dc + 1) * 128],
                            ident[:])
                    nc.scalar.copy(relT[:, dc, rc * 128:(rc + 1) * 128],
                                   ps_t[:])
            psR_cm.__exit__(None, None, None)

            # ---------- P5: extractors -> hsEt/tsEt [emb, n] ----------
            psE_cm = tc.tile_pool(name="psE", bufs=3, space="PSUM")
            psE = psE_cm.__enter__()
            drh_cm = tc.tile_pool(name="drh", bufs=6, space="DRAM")
            drh = drh_cm.__enter__()
            hsDs = []
            tsDs = []
            for ec in range(6):
                for (w, bvec, ew, oh, dst) in (
                    (wh_sb, bh, EWh, ohx, hsEt),
                    (wt_sb, bt, EWt, ohy, tsEt),
                ):
                    ps_e = psE.tile([128, RPC], F32, tag="sm", name="ps_e")
                    self.mm(ps_e[:], ew[:, ec * 128:(ec + 1) * 128], oh[:],
                            start=True, stop=False)
                    for kc in range(8, 16):
                        self.mm(ps_e[:], w[:, kc, ec * 128:(ec + 1) * 128],
                                relT[:, kc % 8, :],
                                start=False, stop=(kc == 15))
                    nc.scalar.activation(dst[:, ec, :], ps_e[:], AF.Tanh,
                                         bias=bvec[:, ec:ec + 1])
                # stage hs/ts chunks to DRAM for the P6 broadcast chain
                hsD = drh.tile([128, RPC], fdt, tag="hsD", name=f"hsD{ec}")
                nc.sync.dma_start(hsD[:], hsEt[:, ec, :])
                hsDs.append(hsD)
                tsD = drh.tile([128, RPC], fdt, tag="tsD", name=f"tsD{ec}")
                nc.sync.dma_start(tsD[:], tsEt[:, ec, :])
                tsDs.append(tsD)
            psE_cm.__exit__(None, None, None)
            wpool_cm.__exit__(None, None, None)

            # ---------- P6: factored bl products + classifier GEMM -------
            with (
                tc.tile_pool(name="stp", bufs=2) as stp,
                tc.tile_pool(name="blp", bufs=2) as blp,
                tc.tile_pool(name="dramp", bufs=2, space="DRAM") as dramp,
                tc.tile_pool(name="ps_lt", bufs=1, space="PSUM") as ps_lt,
            ):
                pslt = ps_lt.tile([NCL, RPC], F32)
                wb_late = []
                for k in range(NWB, K12):
                    wbl = blp.tile([128, 32 * NCL], fdt, tag=f"wbl{k}",
                                   bufs=1, name=f"wbl{k}")
                    nc.sync.dma_start(wbl[:], wbv[:, k, :])
                    wb_late.append(wbl)
                for k in range(K12):
                    ec, kk = k // 2, 64 * (k % 2)
                    hsS = stp.tile([128, NI, RPC], fdt, tag="hs", name="hsS")
                    tsS = stp.tile([128, NJ, RPC], fdt, tag="ts", name="tsS")
                    # hs: D2D broadcast replicate (gpsimd) + plain read
                    # (scalar).  hsS[p, c, :] = hs[kk + 8*(p%8) + c]
                    hsR = dramp.tile([128, NI * RPC], fdt, tag="hsR",
                                     name="hsR")
                    nc.gpsimd.dma_start(
                        hsR[:].rearrange("(j a) x -> j a x", j=16),
                        hsDs[ec][kk:kk + 64, :]
                        .rearrange("(a c) n -> a (c n)", a=8)
                        .unsqueeze(0).broadcast_to([16, 8, NI * RPC]))
                    nc.scalar.dma_start(
                        hsS[:], hsR[:].rearrange("p (c n) -> p c n", c=NI))
                    # ts: same D2D broadcast chain.
                    # tsS[p, m, :] = ts[kk + 4*(p//8) + m]
                    tsR = dramp.tile([128, NJ * RPC], fdt, tag="tsR",
                                     name="tsR")
                    nc.gpsimd.dma_start(
                        tsR[:].rearrange("(j a) x -> a j x", j=16),
                        tsDs[ec][kk:kk + 64, :]
                        .rearrange("(j m) n -> j (m n)", j=16)
                        .unsqueeze(0).broadcast_to([8, 16, NJ * RPC]))
                    nc.scalar.dma_start(
                        tsS[:], tsR[:].rearrange("p (m n) -> p m n", m=NJ))
                    if self.debug and k == 0:
                        nc.scalar.dma_start(d["dbg_hss"].ap(), hsS[:])
                        nc.scalar.dma_start(d["dbg_tss"].ap(), tsS[:])
                    blT = blp.tile([128, NI, NJ, RPC], fdt, tag="blT",
                                   name="blT")
                    in0 = hsS[:].unsqueeze(2).broadcast_to([128, NI, NJ, RPC])
                    in1 = tsS[:].unsqueeze(1).broadcast_to([128, NI, NJ, RPC])
                    if k in P6_POOL_KS:
                        nc.gpsimd.scalar_tensor_tensor(
                            blT[:], in0, 1.0, in1,
                            op0=mybir.AluOpType.mult, op1=mybir.AluOpType.mult)
                    else:
                        for ch in range(2):
                            nc.vector.tensor_mul(
                                blT[:, 4 * ch:4 * (ch + 1), :, :],
                                in0[:, 4 * ch:4 * (ch + 1), :, :],
                                in1[:, 4 * ch:4 * (ch + 1), :, :])
                    if self.debug and k == 0:
                        nc.scalar.dma_start(d["dbg_bl"].ap(), blT[:])
                    for c in range(NI):
                        for m in range(NJ):
                            q = c * NJ + m
                            cg = k * 32 + q
                            wbk = (wb_sb[:, k, q * NCL:(q + 1) * NCL]
                                   if k < NWB else
                                   wb_late[k - NWB][:, q * NCL:(q + 1) * NCL])
                            self.mm(pslt[:], wbk, blT[:, c, m, :],
                                    start=(cg == 0), stop=(cg == 383))

                out_sb = mid.tile([NCL, RPC], F32, name="out_sb")
                nc.scalar.activation(out_sb[:], pslt[:], AF.Identity,
                                     bias=bbc[:])
                nc.sync.dma_start(d["lt"].ap(), out_sb[:])
                if self.debug:
                    nc.sync.dma_start(d["dbg_ct"].ap(), CTmm[:])
                    nc.sync.dma_start(d["dbg_hs"].ap(), hsEt[:])
                    nc.sync.dma_start(d["dbg_ts"].ap(), tsEt[:])
                    nc.sync.dma_start(d["dbg_rel"].ap(), relT[:])
            drh_cm.__exit__(None, None, None)
            wbp_cm.__exit__(None, None, None)


_PROGRAM_CACHE = {}


def _get_program(mm_mode: str):
    if mm_mode not in _PROGRAM_CACHE:
        _PROGRAM_CACHE[mm_mode] = _Builder(mm_mode)
    return _PROGRAM_CACHE[mm_mode]


def _host_inputs(seq_lhs, ent_lhs, ent_to_seq_attn, entity_id_labels, hts,
                 Wh, bh, Wt, bt, Wb, bb):
    """Build the 8 per-core input maps (all host-side numpy)."""
    import ml_dtypes

    fdt = np.dtype(ml_dtypes.bfloat16)
    seq_lhs = np.asarray(seq_lhs, np.float32)
    ent_lhs = np.asarray(ent_lhs, np.float32)
    ent_to_seq_attn = np.asarray(ent_to_seq_attn, np.float32)
    entity_id_labels = np.asarray(entity_id_labels)
    hts = np.asarray(hts)
    Wh = np.asarray(Wh, np.float32)
    Wt = np.asarray(Wt, np.float32)
    Wb = np.asarray(Wb, np.float32)
    bh = np.asarray(bh, np.float32)
    bt = np.asarray(bt, np.float32)
    bb = np.asarray(bb, np.float32)

    # device chunk (k, q=(c,m)) row p maps to Wb row k*4096 + 64*i + j with
    # i = 8c + p//16, j = 16m + p%16
    p_ = np.arange(128)
    c_ = np.arange(NI)
    m_ = np.arange(NJ)
    k_ = np.arange(K12)
    i_ = (8 * (p_[None, None, None, :] % 8)
          + c_[None, :, None, None])                  # [1, c, 1, p]
    j_ = (4 * (p_[None, None, None, :] // 8)
          + m_[None, None, :, None])                  # [1, 1, m, p]
    rows = (k_[:, None, None, None] * 4096 + 64 * i_ + j_)  # [k, c, m, p]
    wb_r = np.ascontiguousarray(
        Wb[rows.reshape(-1), :].reshape(K12 * 32, 128, NCL)
        .transpose(1, 0, 2).reshape(128, 384 * NCL)
    ).astype(fdt)
    wh_c = np.ascontiguousarray(
        Wh.reshape(16, 128, EMB).transpose(1, 0, 2).reshape(128, 16 * EMB)
    ).astype(fdt)
    wt_c = np.ascontiguousarray(
        Wt.reshape(16, 128, EMB).transpose(1, 0, 2).reshape(128, 16 * EMB)
    ).astype(fdt)
    bh_c = np.ascontiguousarray(bh.reshape(EMB // 128, 128).T)
    bt_c = np.ascontiguousarray(bt.reshape(EMB // 128, 128).T)
    bb_c = np.ascontiguousarray(bb.reshape(NCL, 1))
    ident = np.eye(128, dtype=np.float32).astype(fdt)

    in_maps = []
    for c in range(NCORES):
        doc, half = divmod(c, 2)
        sl = slice(half * RPC, (half + 1) * RPC)
        labels = entity_id_labels[doc].astype(np.int64)
        cnt = np.bincount(labels, minlength=E).astype(np.float32)
        S = np.zeros((M, E), np.float32)
        S[np.arange(M), labels] = 1.0
        smean = S / np.maximum(cnt, 1.0)[None, :]  # [M, E]
        eadd = (cnt == 0).astype(np.float32).reshape(E, 1)
        hi = hts[doc, sl, 0].astype(np.int64)
        ti = hts[doc, sl, 1].astype(np.int64)
        ohx = np.zeros((E, RPC), np.float32)
        ohx[hi, np.arange(RPC)] = 1.0
        ohy = np.zeros((E, RPC), np.float32)
        ohy[ti, np.arange(RPC)] = 1.0
        ohxy2 = np.concatenate([smean @ ohx, smean @ ohy], axis=1)  # [M, 512]
        attn = np.ascontiguousarray(
            ent_to_seq_attn[doc].transpose(1, 0, 2).reshape(M, NH * L)
        ).astype(fdt)
        seq_r = seq_lhs[doc].reshape(8, 128, L).transpose(1, 0, 2)
        seq_aug = np.concatenate(
            [seq_r, np.ones((128, 8, 1), np.float32)], axis=2
        )
        in_maps.append({
            "ent": np.ascontiguousarray(ent_lhs[doc]),
            "attn": attn,
            "seq": np.ascontiguousarray(
                seq_aug.reshape(128, 8 * (L + 1))).astype(fdt),
            "ssum": S.astype(fdt),
            "ohxy2": ohxy2.astype(fdt),
            "eadd": eadd,
            "ohx": ohx.astype(fdt),
            "ohy": ohy.astype(fdt),
            "wh": wh_c, "wt": wt_c, "bh": bh_c, "bt": bt_c,
            "wb": wb_r, "bbc": bb_c, "ident": ident,
        })
    return in_maps


_LAST_RESULTS = {}


def kernel(**inputs) -> np.ndarray:
    prog = _get_program(MM_MODE)
    in_maps = _host_inputs(**inputs)
    trace = os.environ.get("DOCRED_TRACE", "0") == "1"
    res = run_bass_kernel_spmd(
        prog.nc, in_maps, core_ids=list(range(NCORES)), trace=trace,
    )
    _LAST_RESULTS["res"] = res
    out = np.empty((B * R, NCL), np.float32)
    for c in range(NCORES):
        doc, half = divmod(c, 2)
        lt = res.results[c]["lt"]  # [NCL, RPC]
        out[doc * R + half * RPC: doc * R + (half + 1) * RPC, :] = lt.T
    return out
